# revision 10
# baseline (speedup 1.0000x reference)
"""Trainium2 Bass kernel for ChannelAttention.

    k      = einsum('bcit,i->bct', signals, alpha)          # [B, C, T]
    scores = einsum('bct,ts,bds->bcd', k, Wc, k)            # [B, C, C]
    att    = softmax(scores, axis=-1)
    out    = einsum('bci,bint->bcnt', att, signals)         # [B, C, N, T]

Sharding: data-parallel over batch B=16 across 8 cores (2 batch elements per
core); Wc/alpha replicated; no collectives.

Per-core program (batches b0, b1 packed into 128 partitions as (b, c) pairs):
  Phase A: kT[t, b*64+c] accumulated on PE: lhsT = sig[b, c, i-chunk, :]
           ([i, t] natural layout), rhs = alpha chunk [i, 1]; 4-chunk PSUM
           accumulation per column.
  Phase B: kWT = (lhsT=Wc) @ kT; scores = (lhsT=kWT) @ kT (block-diag valid);
           row softmax per 64x64 batch block; PE transpose -> attT.
  Phase C: out tiles = (lhsT=attT) @ sig tiles [(b c), (n t)-chunk] streamed,
           PSUM -> SBUF -> HBM.

Built on bacc.Bacc: its compile() pass splits multi-semaphore waits into
event-semaphore instructions (TRN2 allows only 1 wait per instruction).
"""

import numpy as np
from contextlib import ExitStack

import concourse.bass as bass
import concourse.bacc as bacc
import concourse.tile as tile
import concourse.mybir as mybir
from concourse.bass_utils import run_bass_kernel_spmd
from concourse.masks import make_identity

B, C, N, T = 16, 64, 512, 128
NCORES = 8
BPC = B // NCORES          # batches per core
P = 128
NT = N * T                 # 65536 free elements per (b, c) channel
F = 2048                   # phase-C free chunk (8 KiB / partition per tile)
MM_F = 512                 # fp32 matmul moving-operand max free dim
FP32 = mybir.dt.float32
F32R = mybir.dt.float32r

_PROGRAM_CACHE = {}


def _build_program() -> bass.Bass:
    nc = bacc.Bacc(None)
    sig_h = nc.declare_dram_parameter("signals", [BPC, C, N, T], FP32, isOutput=False)
    wc_h = nc.declare_dram_parameter("Wc", [T, T], FP32, isOutput=False)
    al_h = nc.declare_dram_parameter("alpha", [N], FP32, isOutput=False)
    out_h = nc.declare_dram_parameter("out", [BPC, C, N, T], FP32, isOutput=True)

    sig = sig_h.ap()
    out = out_h.ap()
    sig_flat = sig.rearrange("b c n t -> (b c) (n t)")
    out_flat = out.rearrange("b c n t -> (b c) (n t)")

    with ExitStack() as ctx:
        tc = ctx.enter_context(tile.TileContext(nc))
        singles = ctx.enter_context(tc.tile_pool(name="singles", bufs=1))
        apool = ctx.enter_context(tc.tile_pool(name="apool", bufs=2))
        cpool = ctx.enter_context(tc.tile_pool(name="cpool", bufs=4))
        opool = ctx.enter_context(tc.tile_pool(name="opool", bufs=4))
        small = ctx.enter_context(tc.tile_pool(name="small", bufs=1))
        pk = ctx.enter_context(tc.tile_pool(name="pk", bufs=1, space="PSUM"))
        pb = ctx.enter_context(tc.tile_pool(name="pb", bufs=2, space="PSUM"))
        po = ctx.enter_context(tc.tile_pool(name="po", bufs=5, space="PSUM"))

        # --- constants
        wc_sb = singles.tile([T, T], FP32)
        nc.sync.dma_start(out=wc_sb, in_=wc_h.ap())
        alpha_sb = singles.tile([P, N // P], FP32)
        nc.sync.dma_start(out=alpha_sb, in_=al_h.ap().rearrange("(o p) -> p o", p=P))
        ident = singles.tile([P, P], FP32)
        make_identity(nc, ident)

        # --- Phase A: kT [t, (b c)]
        kt_ps = pk.tile([P, P], FP32)
        n_ichunks = N // P
        for b in range(BPC):
            for ic in range(n_ichunks):
                a_tile = apool.tile([P, C, T], FP32, tag="a")
                nc.sync.dma_start(
                    out=a_tile,
                    in_=sig[b, :, ic * P:(ic + 1) * P, :].rearrange("c i t -> i c t"),
                )
                for c in range(C):
                    col = b * C + c
                    # start=True clears has_written for the WHOLE bank, so only
                    # the very first matmul may set it; later columns' first
                    # writes overwrite via per-element has_written=0.
                    nc.tensor.matmul(
                        kt_ps[:, col:col + 1],
                        lhsT=a_tile[:, c, :],
                        rhs=alpha_sb[:, ic:ic + 1],
                        start=(b == 0 and ic == 0 and c == 0),
                        stop=(b == BPC - 1 and ic == n_ichunks - 1 and c == C - 1),
                    )
        kt_sb = small.tile([P, P], FP32)
        nc.vector.tensor_copy(kt_sb, kt_ps)

        # --- Phase B: scores + softmax + transpose
        kwt_ps = pb.tile([P, P], FP32, tag="pb")
        nc.tensor.matmul(kwt_ps, lhsT=wc_sb, rhs=kt_sb, start=True, stop=True)
        kwt_sb = small.tile([P, P], FP32)
        nc.vector.tensor_copy(kwt_sb, kwt_ps)

        sc_ps = pb.tile([P, P], FP32, tag="pb")
        nc.tensor.matmul(sc_ps, lhsT=kwt_sb, rhs=kt_sb, start=True, stop=True)

        att = small.tile([P, P], FP32)
        nc.scalar.memzero(att)
        mx = small.tile([P, 1], FP32)
        nmx = small.tile([P, 1], FP32)
        ssum = small.tile([P, 1], FP32)
        rsum = small.tile([P, 1], FP32)
        for b in range(BPC):
            rows = slice(b * C, (b + 1) * C)
            cols = slice(b * C, b * C + C)
            blk = sc_ps[rows, cols]
            nc.vector.reduce_max(out=mx[rows], in_=blk, axis=mybir.AxisListType.X)
            nc.vector.tensor_scalar_mul(nmx[rows], mx[rows], -1.0)
            nc.scalar.activation(
                att[rows, cols], blk, mybir.ActivationFunctionType.Exp,
                bias=nmx[rows], scale=1.0, accum_out=ssum[rows],
            )
        nc.vector.reciprocal(rsum, ssum)
        for b in range(BPC):
            rows = slice(b * C, (b + 1) * C)
            cols = slice(b * C, b * C + C)
            nc.scalar.mul(att[rows, cols], att[rows, cols], rsum[rows])

        attt_ps = pb.tile([P, P], FP32, tag="pb")
        nc.tensor.transpose(attt_ps, att, ident)
        attt_sb = small.tile([P, P], F32R)
        nc.vector.tensor_copy(attt_sb, attt_ps)

        # --- Phase C: out = (lhsT=attT) @ sig, streamed over (n t)
        for f in range(NT // F):
            c_tile = cpool.tile([P, F], F32R, tag="c")
            nc.sync.dma_start(
                out=c_tile, in_=sig_flat[:, f * F:(f + 1) * F].bitcast(F32R)
            )
            o_tile = opool.tile([P, F], FP32, tag="o")
            for j in range(F // MM_F):
                o_ps = po.tile([P, MM_F], FP32, tag="po")
                nc.tensor.matmul(
                    o_ps, lhsT=attt_sb, rhs=c_tile[:, j * MM_F:(j + 1) * MM_F],
                    start=True, stop=True,
                )
                nc.vector.tensor_copy(o_tile[:, j * MM_F:(j + 1) * MM_F], o_ps)
            nc.scalar.dma_start(out=out_flat[:, f * F:(f + 1) * F], in_=o_tile)

    nc.compile()
    return nc


def _get_program() -> bass.Bass:
    if "nc" not in _PROGRAM_CACHE:
        _PROGRAM_CACHE["nc"] = _build_program()
    return _PROGRAM_CACHE["nc"]


def kernel(signals, Wc, alpha, **run_kwargs):
    signals = np.ascontiguousarray(np.asarray(signals, dtype=np.float32))
    Wc = np.ascontiguousarray(np.asarray(Wc, dtype=np.float32))
    alpha = np.ascontiguousarray(np.asarray(alpha, dtype=np.float32))
    assert signals.shape == (B, C, N, T)

    nc = _get_program()
    core_ids = list(range(NCORES))
    in_maps = [
        {
            "signals": signals[j * BPC:(j + 1) * BPC],
            "Wc": Wc,
            "alpha": alpha,
        }
        for j in range(NCORES)
    ]
    res = run_bass_kernel_spmd(nc, in_maps, core_ids, **run_kwargs)
    out = np.empty((B, C, N, T), dtype=np.float32)
    for j in range(NCORES):
        out[j * BPC:(j + 1) * BPC] = res.results[j]["out"]
    if run_kwargs:
        kernel.last_results = res
    return out


# revision 11
# speedup vs baseline: 1.3001x; 1.3001x over previous
"""Trainium2 Bass kernel for ChannelAttention.

    k      = einsum('bcit,i->bct', signals, alpha)          # [B, C, T]
    scores = einsum('bct,ts,bds->bcd', k, Wc, k)            # [B, C, C]
    att    = softmax(scores, axis=-1)
    out    = einsum('bci,bint->bcnt', att, signals)         # [B, C, N, T]

Sharding: data-parallel over batch B=16 across 8 cores (2 batch elements per
core); Wc/alpha replicated; no collectives.

Per-core program (batches b0, b1 packed into 128 partitions as (b, c) pairs):
  Phase A: kT[t, b*64+c] accumulated on PE: lhsT = sig[b, c, i-chunk, :]
           ([i, t] natural layout), rhs = alpha chunk [i, 1]; 4-chunk PSUM
           accumulation per column.
  Phase B: kWT = (lhsT=Wc) @ kT; scores = (lhsT=kWT) @ kT (block-diag valid);
           row softmax per 64x64 batch block; PE transpose -> attT.
  Phase C: out tiles = (lhsT=attT) @ sig tiles [(b c), (n t)-chunk] streamed,
           PSUM -> SBUF -> HBM.

Built on bacc.Bacc: its compile() pass splits multi-semaphore waits into
event-semaphore instructions (TRN2 allows only 1 wait per instruction).
"""

import numpy as np
from contextlib import ExitStack

import concourse.bass as bass
import concourse.bacc as bacc
import concourse.tile as tile
import concourse.mybir as mybir
from concourse.bass_utils import run_bass_kernel_spmd
from concourse.masks import make_identity

B, C, N, T = 16, 64, 512, 128
NCORES = 8
BPC = B // NCORES          # batches per core
P = 128
NT = N * T                 # 65536 free elements per (b, c) channel
F = 2048                   # phase-C free chunk (8 KiB / partition per tile)
MM_F = 512                 # fp32 matmul moving-operand max free dim
FP32 = mybir.dt.float32
F32R = mybir.dt.float32r

_PROGRAM_CACHE = {}


def _build_program() -> bass.Bass:
    nc = bacc.Bacc(None)
    sig_h = nc.declare_dram_parameter("signals", [BPC, C, N, T], FP32, isOutput=False)
    wc_h = nc.declare_dram_parameter("Wc", [T, T], FP32, isOutput=False)
    al_h = nc.declare_dram_parameter("alpha", [N], FP32, isOutput=False)
    out_h = nc.declare_dram_parameter("out", [BPC, C, N, T], FP32, isOutput=True)

    sig = sig_h.ap()
    out = out_h.ap()
    sig_flat = sig.rearrange("b c n t -> (b c) (n t)")
    out_flat = out.rearrange("b c n t -> (b c) (n t)")

    with ExitStack() as ctx:
        tc = ctx.enter_context(tile.TileContext(nc))
        singles = ctx.enter_context(tc.tile_pool(name="singles", bufs=1))
        apool = ctx.enter_context(tc.tile_pool(name="apool", bufs=2))
        cpool = ctx.enter_context(tc.tile_pool(name="cpool", bufs=4))
        opool = ctx.enter_context(tc.tile_pool(name="opool", bufs=4))
        small = ctx.enter_context(tc.tile_pool(name="small", bufs=1))
        pk = ctx.enter_context(tc.tile_pool(name="pk", bufs=1, space="PSUM"))
        pb = ctx.enter_context(tc.tile_pool(name="pb", bufs=1, space="PSUM"))
        po = ctx.enter_context(tc.tile_pool(name="po", bufs=3, space="PSUM"))

        # --- constants
        wc_sb = singles.tile([T, T], FP32)
        nc.sync.dma_start(out=wc_sb, in_=wc_h.ap())
        alpha_sb = singles.tile([P, N // P], F32R)
        nc.sync.dma_start(
            out=alpha_sb,
            in_=al_h.ap().rearrange("(o p) -> p o", p=P).bitcast(F32R),
        )
        ident = singles.tile([P, P], FP32)
        make_identity(nc, ident)

        # --- Phase A: k rows on partition 0 via alpha-stationary f32r matmuls
        # (M=1, free=512), then kT assembly via PE transposes of [1,128] blocks.
        QC = 16               # channels per psum block: [1, QC*T] = 4 banks
        NQ = C // QC
        n_ichunks = N // P
        k_sb = small.tile([1, BPC * C * T], FP32)
        for b in range(BPC):
            for q in range(NQ):
                kr_ps = pk.tile([1, QC * T], FP32, tag="kr")
                for ic in range(n_ichunks):
                    a_sub = apool.tile([P, QC, T], F32R, tag="a")
                    nc.sync.dma_start(
                        out=a_sub,
                        in_=sig[b, q * QC:(q + 1) * QC, ic * P:(ic + 1) * P, :]
                        .rearrange("c i t -> i c t").bitcast(F32R),
                    )
                    a_flat = a_sub.rearrange("i c t -> i (c t)")
                    for j in range(QC * T // MM_F):
                        nc.tensor.matmul(
                            kr_ps[:, j * MM_F:(j + 1) * MM_F],
                            lhsT=alpha_sb[:, ic:ic + 1],
                            rhs=a_flat[:, j * MM_F:(j + 1) * MM_F],
                            start=(ic == 0),
                            stop=(ic == n_ichunks - 1),
                        )
                nc.vector.tensor_copy(
                    k_sb[:, (b * C + q * QC) * T:(b * C + (q + 1) * QC) * T],
                    kr_ps,
                )

        kt_ps = pb.tile([P, P], FP32, tag="pb")
        for col in range(P):
            nc.tensor.transpose(
                kt_ps[:, col:col + 1],
                k_sb[:, col * T:(col + 1) * T],
                ident[0:1, 0:1],
            )
        kt_sb = small.tile([P, P], FP32)
        nc.vector.tensor_copy(kt_sb, kt_ps)

        # --- Phase B: scores + softmax + transpose
        kwt_ps = pb.tile([P, P], FP32, tag="pb")
        nc.tensor.matmul(kwt_ps, lhsT=wc_sb, rhs=kt_sb, start=True, stop=True)
        kwt_sb = small.tile([P, P], FP32)
        nc.vector.tensor_copy(kwt_sb, kwt_ps)

        sc_ps = pb.tile([P, P], FP32, tag="pb")
        nc.tensor.matmul(sc_ps, lhsT=kwt_sb, rhs=kt_sb, start=True, stop=True)

        att = small.tile([P, P], FP32)
        nc.scalar.memzero(att)
        mx = small.tile([P, 1], FP32)
        nmx = small.tile([P, 1], FP32)
        ssum = small.tile([P, 1], FP32)
        rsum = small.tile([P, 1], FP32)
        for b in range(BPC):
            rows = slice(b * C, (b + 1) * C)
            cols = slice(b * C, b * C + C)
            blk = sc_ps[rows, cols]
            nc.vector.reduce_max(out=mx[rows], in_=blk, axis=mybir.AxisListType.X)
            nc.vector.tensor_scalar_mul(nmx[rows], mx[rows], -1.0)
            nc.scalar.activation(
                att[rows, cols], blk, mybir.ActivationFunctionType.Exp,
                bias=nmx[rows], scale=1.0, accum_out=ssum[rows],
            )
        nc.vector.reciprocal(rsum, ssum)
        for b in range(BPC):
            rows = slice(b * C, (b + 1) * C)
            cols = slice(b * C, b * C + C)
            nc.scalar.mul(att[rows, cols], att[rows, cols], rsum[rows])

        attt_ps = pb.tile([P, P], FP32, tag="pb")
        nc.tensor.transpose(attt_ps, att, ident)
        attt_sb = small.tile([P, P], F32R)
        nc.vector.tensor_copy(attt_sb, attt_ps)

        # --- Phase C: out = (lhsT=attT) @ sig, streamed over (n t)
        for f in range(NT // F):
            c_tile = cpool.tile([P, F], F32R, tag="c")
            nc.sync.dma_start(
                out=c_tile, in_=sig_flat[:, f * F:(f + 1) * F].bitcast(F32R)
            )
            o_tile = opool.tile([P, F], FP32, tag="o")
            for j in range(F // MM_F):
                o_ps = po.tile([P, MM_F], FP32, tag="po")
                nc.tensor.matmul(
                    o_ps, lhsT=attt_sb, rhs=c_tile[:, j * MM_F:(j + 1) * MM_F],
                    start=True, stop=True,
                )
                nc.vector.tensor_copy(o_tile[:, j * MM_F:(j + 1) * MM_F], o_ps)
            nc.scalar.dma_start(out=out_flat[:, f * F:(f + 1) * F], in_=o_tile)

    nc.compile()
    return nc


def _get_program() -> bass.Bass:
    if "nc" not in _PROGRAM_CACHE:
        _PROGRAM_CACHE["nc"] = _build_program()
    return _PROGRAM_CACHE["nc"]


def kernel(signals, Wc, alpha, **run_kwargs):
    signals = np.ascontiguousarray(np.asarray(signals, dtype=np.float32))
    Wc = np.ascontiguousarray(np.asarray(Wc, dtype=np.float32))
    alpha = np.ascontiguousarray(np.asarray(alpha, dtype=np.float32))
    assert signals.shape == (B, C, N, T)

    nc = _get_program()
    core_ids = list(range(NCORES))
    in_maps = [
        {
            "signals": signals[j * BPC:(j + 1) * BPC],
            "Wc": Wc,
            "alpha": alpha,
        }
        for j in range(NCORES)
    ]
    res = run_bass_kernel_spmd(nc, in_maps, core_ids, **run_kwargs)
    out = np.empty((B, C, N, T), dtype=np.float32)
    for j in range(NCORES):
        out[j * BPC:(j + 1) * BPC] = res.results[j]["out"]
    if run_kwargs:
        kernel.last_results = res
    return out
